# revision 31
# baseline (speedup 1.0000x reference)
"""AnyLoc/NetVLAD pooling kernel for 8 Trainium2 NeuronCores.

Full inputs in, full output out. Internally:
  - data-parallel over batch: core r owns samples {2r, 2r+1}
  - comp_w sharded over its OUT dim: core r owns output columns [256r, 256r+256)
  - per-sample AllGathers of the tiny VLAD vectors (overlapped with compute),
    final row-norm via a 64-byte AllReduce; host concatenates output slices.

Hardcoded problem shape: N=16, T=2048, D=1024, P=256, K=64, OUT=2048 (f32).
"""

import math
import sys
import types

import numpy as np

N_CORES = 8
N, T, D, P, K, OUT = 16, 2048, 1024, 256, 64, 2048
SPC = N // N_CORES          # samples per core = 2
TT = SPC * T                # tokens per core = 4096
NT = TT // 128              # 128-token tiles per core = 32
NTS = T // 128              # tiles per sample = 16
OSL = OUT // N_CORES        # output slice per core = 256
F = K * P                   # flattened VLAD dim = 16384
FC = F // 128               # f-chunks = 128
DC = D // 128               # d-chunks = 8
PC = P // 128               # p-chunks = 2
HZ = P + K                  # fused h|z matmul width = 320


def _install_ntff_hook():
    """Make run_bass_kernel_spmd(trace=True) usable in this container: the
    image's antenv stub lacks axon_hooks, so inject one wired to the axon .so.
    Harmless if tracing is never requested."""
    if "antenv.axon_hooks" in sys.modules:
        return
    try:
        from trn_agent_boot.trn_boot import _ntff_profile_via_ctypes

        hook = _ntff_profile_via_ctypes("/opt/axon/libaxon_pjrt.so")
    except Exception:
        hook = None
    mod = types.ModuleType("antenv.axon_hooks")
    mod.get_axon_ntff_profile_hook = lambda: hook
    mod.set_axon_ntff_profile_hook = lambda h: None
    sys.modules["antenv.axon_hooks"] = mod


_NC_CACHE = {}


def _build():
    import concourse.bacc as bacc
    import concourse.mybir as mybir
    import concourse.tile as tile
    from concourse.masks import make_identity

    f32 = mybir.dt.float32
    bf16 = mybir.dt.bfloat16
    Alu = mybir.AluOpType
    Act = mybir.ActivationFunctionType

    nc = bacc.Bacc(
        "TRN2",
        target_bir_lowering=False,
        debug=False,
        enable_asserts=False,
        num_devices=N_CORES,
    )

    # ---- DRAM I/O (per-core shards; names are the in_map keys) ----
    xt_d = nc.dram_tensor("xt", [128, DC * TT], f32, kind="ExternalInput")
    pca_wt_d = nc.dram_tensor("pca_wt", [D, P], f32, kind="ExternalInput")
    pca_w_d = nc.dram_tensor("pca_w", [P, D], f32, kind="ExternalInput")
    pca_b_d = nc.dram_tensor("pca_b", [1, P], f32, kind="ExternalInput")
    pca_bc_d = nc.dram_tensor("pca_b_col", [128, 2], f32, kind="ExternalInput")
    conv_wt_d = nc.dram_tensor("conv_wt", [P, K], f32, kind="ExternalInput")
    conv_bb_d = nc.dram_tensor("conv_b_bc", [128, K], f32, kind="ExternalInput")
    cent_d = nc.dram_tensor("cent", [K, P], f32, kind="ExternalInput")
    comp_wt_d = nc.dram_tensor("comp_wt", [128, FC * OSL], f32, kind="ExternalInput")
    comp_b_d = nc.dram_tensor("comp_b", [1, OSL], f32, kind="ExternalInput")
    out_d = nc.dram_tensor("out", [N, OSL], f32, kind="ExternalOutput")

    rg = [list(range(N_CORES))]
    LN8TH = math.log(0.125)

    with tile.TileContext(nc) as tc:
        with (
            tc.tile_pool(name="consts", bufs=1) as consts,
            tc.tile_pool(name="work", bufs=3) as work,
            tc.tile_pool(name="small", bufs=4) as small,
            tc.tile_pool(name="ph", bufs=2, space="PSUM") as ph_pool,
            tc.tile_pool(name="pagg", bufs=2, space="PSUM") as pagg_pool,
            tc.tile_pool(name="pmisc", bufs=2, space="PSUM") as pmisc_pool,
            tc.tile_pool(name="pout", bufs=2, space="PSUM") as pout_pool,
            tc.tile_pool(name="dram", bufs=1, space="DRAM") as dram,
        ):
            # ---- persistent SBUF tensors ----
            WG_sb = consts.tile([128, DC, HZ], bf16, tag="WG")    # [pca_w.T | G]
            pw_sb = consts.tile([128, PC, D], bf16, tag="pw")     # pca_w
            cwt_sb = consts.tile([128, PC, K], bf16, tag="cwt")   # conv_w.T
            cbb_sb = consts.tile([128, K], f32, tag="cbb")        # conv_b bcast
            cent_sb = consts.tile([K, P], f32, tag="cent")
            bg_sb = consts.tile([1, HZ], bf16, tag="bg")          # [pca_b | g0]
            pcabc_sb = consts.tile([128, 2], bf16, tag="pcabc")
            compb_sb = consts.tile([1, OSL], bf16, tag="compb")
            ones_sb = consts.tile([1, 128], bf16, tag="ones")
            ln8_sb = consts.tile([128, 1], f32, tag="ln8")
            ident_sb = consts.tile([128, 128], f32, tag="ident")
            xt_sb = consts.tile([128, DC, TT], bf16, tag="xt")
            cwT_sb = consts.tile([128, FC, OSL], bf16, tag="cwT")  # comp_w.T
            h_all = consts.tile([128, NT, P + 1], bf16, tag="hall")
            z_all = consts.tile([128, NT, K], f32, tag="zall")
            u_all = consts.tile([128, NT, K], bf16, tag="uall")
            nsq_all = consts.tile([128, NT], f32, tag="nsq")
            lnn_all = consts.tile([128, NT], f32, tag="lnn")
            inv_all = consts.tile([128, NT], f32, tag="inv")
            n_all = consts.tile([128, NT], f32, tag="nall")
            S_all = consts.tile([128, NT], f32, tag="Sall")
            rS_all = consts.tile([128, NT], f32, tag="rSall")
            vT_own = consts.tile([128, SPC, 128], bf16, tag="vTown")
            vT_ev = consts.tile([128, N_CORES, 128], bf16, tag="vTev")
            vT_od = consts.tile([128, N_CORES, 128], bf16, tag="vTod")
            sq_scr = consts.tile([128, P], bf16, tag="sqscr")
            sq64_scr = consts.tile([K, P], bf16, tag="sq64")
            osq_sb = consts.tile([N, 1], f32, tag="osq")
            rno_sb = consts.tile([N, 1], f32, tag="rno")
            rno_od = consts.tile([N_CORES, 1], f32, tag="rnood")

            # DRAM bounce buffers for collectives (f32-typed views of bf16
            # bits: halves the CCE element count -> faster AllGather)
            agv_in = [
                dram.tile([128, 64], f32, tag=f"agi{s}", name=f"agv_in{s}")
                for s in range(SPC)
            ]
            agv_out = [
                dram.tile(
                    [128 * N_CORES, 64], f32, tag=f"ago{s}", name=f"agv_out{s}"
                )
                for s in range(SPC)
            ]
            ar_in = dram.tile([N, 1], f32, tag="ari")
            ar_out = dram.tile([N, 1], f32, tag="aro")

            # ---- const loads ----
            nc.gpsimd.dma_start(
                WG_sb[:, :, 0:P], pca_wt_d.ap().rearrange("(c k) p -> k c p", k=128)
            )
            nc.gpsimd.dma_start(
                pw_sb[:], pca_w_d.ap().rearrange("(c k) d -> k c d", k=128)
            )
            nc.gpsimd.dma_start(
                cwt_sb[:], conv_wt_d.ap().rearrange("(c k) n -> k c n", k=128)
            )
            nc.sync.dma_start(cbb_sb[:], conv_bb_d.ap())
            nc.sync.dma_start(cent_sb[:], cent_d.ap())
            nc.gpsimd.dma_start(bg_sb[0:1, 0:P], pca_b_d.ap())
            nc.gpsimd.dma_start(pcabc_sb[:], pca_bc_d.ap())
            nc.gpsimd.dma_start(compb_sb[:], comp_b_d.ap())
            nc.vector.memset(ones_sb[:], 1.0)
            nc.vector.memset(ln8_sb[:], LN8TH)
            make_identity(nc, ident_sb[:])

            # ---- x loads: token-quarter major so PE can start early.
            # Host pre-tiles xt to [128, dc*TT + t]: every DMA below is one
            # contiguous run per partition (minimal SWDGE descriptor count).
            # Three staged phases with pairwise dep edges so each phase gets
            # the full HBM bandwidth instead of ring round-robin: x(sample 0),
            # then x(sample 1), then comp_w.T.
            from concourse.tile_rust import add_dep_helper

            NQ = 2
            QW = TT // NQ  # one sample's tokens per load block
            xt_re = xt_d.ap().rearrange("k (c t) -> k c t", c=DC)
            xdmas = [[], []]
            for q in range(NQ):
                for dc in range(DC):
                    i = nc.gpsimd.dma_start(
                        xt_sb[:, dc, q * QW : (q + 1) * QW],
                        xt_re[:, dc, q * QW : (q + 1) * QW],
                    )
                    xdmas[q].append(i)
                    if q == 1:
                        add_dep_helper(
                            i.ins, xdmas[0][dc].ins, reason="stage x1 after x0"
                        )
            # ---- comp_w.T stream-in (host pre-tiled the same way) ----
            for g in range(8):
                i = nc.gpsimd.dma_start(
                    cwT_sb[:, g * 16 : (g + 1) * 16, :],
                    comp_wt_d.ap()[:, g * 16 * OSL : (g + 1) * 16 * OSL],
                )
                add_dep_helper(
                    i.ins, xdmas[1][g].ins, reason="stage comp_w after x1"
                )

            # ---- G = pca_w.T @ conv_w.T  [D, K], g0 = pca_b @ conv_w.T ----
            for dc in range(DC):
                pG = pmisc_pool.tile([128, 128], f32, tag="pm")
                for pc in range(PC):
                    nc.tensor.matmul(
                        pG[:, 0:K],
                        pw_sb[:, pc, dc * 128 : (dc + 1) * 128],
                        cwt_sb[:, pc, :],
                        start=(pc == 0),
                        stop=(pc == PC - 1),
                    )
                nc.vector.tensor_copy(WG_sb[:, dc, P:HZ], pG[:, 0:K])
            pg0 = pmisc_pool.tile([128, 128], f32, tag="pm")
            for pc in range(PC):
                nc.tensor.matmul(
                    pg0[0:1, 0:K],
                    pcabc_sb[:, pc : pc + 1],
                    cwt_sb[:, pc, :],
                    start=(pc == 0),
                    stop=(pc == PC - 1),
                )
            nc.vector.tensor_copy(bg_sb[0:1, P:HZ], pg0[0:1, 0:K])

            # ================= per-sample stages, manually interleaved ======
            def pass_a(s):
                """Fused h|z matmuls + copies + row sum-squares for sample s."""
                for ti in range(NTS):
                    t0 = s * T + ti * 128
                    g = s * NTS + ti
                    ph = ph_pool.tile([128, HZ], f32, tag="ph")
                    for dc in range(DC):
                        nc.tensor.matmul(
                            ph[:],
                            xt_sb[:, dc, t0 : t0 + 128],
                            WG_sb[:, dc, :],
                            start=(dc == 0),
                            stop=False,
                        )
                    nc.tensor.matmul(
                        ph[:], ones_sb[0:1, 0:128], bg_sb[:], start=False, stop=True
                    )
                    nc.scalar.copy(h_all[:, g, 0:P], ph[:, 0:P])
                    nc.vector.tensor_copy(z_all[:, g, :], ph[:, P:HZ])
                    nc.vector.scalar_tensor_tensor(
                        sq_scr[:],
                        h_all[:, g, 0:P],
                        1.0,
                        h_all[:, g, 0:P],
                        op0=Alu.mult,
                        op1=Alu.mult,
                        accum_out=nsq_all[:, g : g + 1],
                    )

            def pass_b(s):
                """Batched norms, softmax weights, VLAD agg + gather for s."""
                s16 = slice(s * NTS, (s + 1) * NTS)
                nc.scalar.activation(lnn_all[:, s16], nsq_all[:, s16], Act.Ln)
                nc.scalar.activation(
                    inv_all[:, s16], lnn_all[:, s16], Act.Exp, scale=-0.5
                )
                nc.scalar.activation(
                    n_all[:, s16], lnn_all[:, s16], Act.Exp, scale=0.5
                )
                nc.vector.tensor_copy(h_all[:, s16, P], n_all[:, s16])
                for ti in range(NTS):
                    g = s * NTS + ti
                    nc.vector.scalar_tensor_tensor(
                        z_all[:, g, :],
                        z_all[:, g, :],
                        inv_all[:, g : g + 1],
                        cbb_sb[:],
                        op0=Alu.mult,
                        op1=Alu.add,
                    )
                nc.scalar.activation(u_all[:, s16, :], z_all[:, s16, :], Act.Exp)
                nc.vector.reduce_sum(
                    S_all[:, s16], u_all[:, s16, :], axis=mybir.AxisListType.X
                )
                nc.vector.reciprocal(rS_all[:, s16], S_all[:, s16])
                agg_t = pagg_pool.tile([K, P + 1], f32, tag="agg")
                for ti in range(NTS):
                    g = s * NTS + ti
                    w_t = work.tile([128, K], bf16, tag="w")
                    nc.vector.tensor_scalar(
                        w_t[:],
                        u_all[:, g, :],
                        rS_all[:, g : g + 1],
                        inv_all[:, g : g + 1],
                        op0=Alu.mult,
                        op1=Alu.mult,
                    )
                    nc.tensor.matmul(
                        agg_t[:],
                        w_t[:],
                        h_all[:, g, :],
                        start=(ti == 0),
                        stop=(ti == NTS - 1),
                    )

                # per-sample VLAD: subtract centroids, intra-normalize
                agg_sb = work.tile([K, P + 1], f32, tag="aggsb")
                nc.vector.tensor_copy(agg_sb[:], agg_t[:])
                vlneg = work.tile([K, P], f32, tag="vlneg")
                nc.vector.scalar_tensor_tensor(
                    vlneg[:],
                    cent_sb[:],
                    agg_sb[:, P : P + 1],
                    agg_sb[:, 0:P],
                    op0=Alu.mult,
                    op1=Alu.subtract,
                )
                vsq = small.tile([K, 1], f32, tag="vsq")
                nc.vector.scalar_tensor_tensor(
                    sq64_scr[:],
                    vlneg[:],
                    1.0,
                    vlneg[:],
                    op0=Alu.mult,
                    op1=Alu.mult,
                    accum_out=vsq[:],
                )
                lnv = small.tile([K, 1], f32, tag="lnv")
                nc.scalar.activation(lnv[:], vsq[:], Act.Ln)
                r_t = small.tile([K, 1], f32, tag="r")
                nc.scalar.activation(
                    r_t[:], lnv[:], Act.Exp, scale=-0.5, bias=ln8_sb[0:K, :]
                )
                vn = work.tile([K, P], f32, tag="vn")
                nc.vector.tensor_scalar(
                    vn[:], vlneg[:], r_t[:], -1.0, op0=Alu.mult, op1=Alu.mult
                )
                v128 = work.tile([128, 128], f32, tag="v128")
                nc.scalar.dma_start(v128[0 : 2 * K : 2, :], vn[:, 0:128])
                nc.scalar.dma_start(v128[1 : 2 * K : 2, :], vn[:, 128:256])
                ptr = pmisc_pool.tile([128, 128], f32, tag="pm")
                nc.tensor.transpose(ptr[:], v128[:], ident_sb[:])
                nc.vector.tensor_copy(vT_own[:, s, :], ptr[:])

                # AllGather this sample's VLAD across cores (as f32 bits)
                nc.sync.dma_start(agv_in[s][:], vT_own[:, s, :].bitcast(f32))
                nc.gpsimd.collective_compute(
                    "AllGather",
                    Alu.bypass,
                    replica_groups=rg,
                    ins=[agv_in[s].opt()],
                    outs=[agv_out[s].opt()],
                )
                vT_dst = vT_ev if s == 0 else vT_od
                nc.sync.dma_start(
                    vT_dst[:],
                    agv_out[s][:].bitcast(bf16).rearrange(
                        "(b k) c -> k b c", b=N_CORES
                    ),
                )

            pout_t = [None, None]

            def gemm_half(s):
                """out rows of parity s for this core's 256 output columns."""
                vT_dst = vT_ev if s == 0 else vT_od
                pout_t[s] = pout_pool.tile(
                    [N_CORES, OSL], f32, tag="po", name=f"pout{s}"
                )
                for c in range(FC):
                    nc.tensor.matmul(
                        pout_t[s][:],
                        vT_dst[:, :, c],
                        cwT_sb[:, c, :],
                        start=(c == 0),
                        stop=False,
                    )
                nc.tensor.matmul(
                    pout_t[s][:],
                    ones_sb[0:1, 0:N_CORES],
                    compb_sb[:],
                    start=False,
                    stop=True,
                )

            pass_a(0)
            pass_b(0)          # ends with AllGather of sample-0 VLADs
            pass_a(1)          # PE busy here while AG0 completes
            pass_b(1)
            gemm_half(0)
            gemm_half(1)

            # ---- final row norm: AllReduce partial sum-squares (64 B) ----
            out_sl = [None, None]
            for s in range(SPC):
                out_sl[s] = work.tile(
                    [N_CORES, OSL], f32, tag=f"osl{s}", name=f"out_sl{s}"
                )
                nc.vector.tensor_copy(out_sl[s][:], pout_t[s][:])
                osqp = small.tile([N_CORES, 1], f32, tag=f"osqp{s}")
                nc.vector.scalar_tensor_tensor(
                    sq_scr[0:N_CORES, 0:OSL],
                    out_sl[s][:],
                    1.0,
                    out_sl[s][:],
                    op0=Alu.mult,
                    op1=Alu.mult,
                    accum_out=osqp[:],
                )
                nc.sync.dma_start(ar_in[s * N_CORES : (s + 1) * N_CORES, :], osqp[:])
            nc.gpsimd.collective_compute(
                "AllReduce",
                Alu.add,
                replica_groups=rg,
                ins=[ar_in.opt()],
                outs=[ar_out.opt()],
            )
            nc.sync.dma_start(osq_sb[:], ar_out[:])
            lno = small.tile([N, 1], f32, tag="lno")
            nc.scalar.activation(lno[:], osq_sb[:], Act.Ln)
            nc.scalar.activation(rno_sb[:], lno[:], Act.Exp, scale=-0.5)
            nc.sync.dma_start(rno_od[:], rno_sb[N_CORES:N, :])
            for s in range(SPC):
                scal = rno_sb[0:N_CORES, :] if s == 0 else rno_od[:]
                of = work.tile([N_CORES, OSL], f32, tag=f"of{s}")
                nc.vector.tensor_scalar(
                    of[:], out_sl[s][:], scal, None, op0=Alu.mult
                )
                nc.sync.dma_start(out_d.ap()[s : N : SPC, :], of[:])

    nc.compile()
    return nc


def _get_nc():
    if "nc" not in _NC_CACHE:
        _install_ntff_hook()
        _NC_CACHE["nc"] = _build()
    return _NC_CACHE["nc"]


def kernel(**inputs):
    x = np.asarray(inputs["x"], dtype=np.float32)
    pca_w = np.asarray(inputs["pca_w"], dtype=np.float32)
    pca_b = np.asarray(inputs["pca_b"], dtype=np.float32)
    conv_w = np.asarray(inputs["conv_w"], dtype=np.float32)
    conv_b = np.asarray(inputs["conv_b"], dtype=np.float32)
    cent = np.asarray(inputs["centroids"], dtype=np.float32)
    comp_w = np.asarray(inputs["comp_w"], dtype=np.float32)
    comp_b = np.asarray(inputs["comp_b"], dtype=np.float32)

    nc = _get_nc()
    from concourse.bass_utils import run_bass_kernel_spmd

    # host-side layout prep (pure slicing / transposition)
    pca_wt = np.ascontiguousarray(pca_w.T)                      # [D, P]
    pca_b_row = np.ascontiguousarray(pca_b.reshape(1, P))
    pca_b_col = np.ascontiguousarray(pca_b.reshape(2, 128).T)   # [128, 2]
    conv_wt = np.ascontiguousarray(conv_w.T)                    # [P, K]
    conv_b_bc = np.ascontiguousarray(np.broadcast_to(conv_b, (128, K)))
    xt = x.transpose(0, 2, 1)                                   # [N, D, T]

    in_maps = []
    for r in range(N_CORES):
        xt_r = np.concatenate([xt[SPC * r + j] for j in range(SPC)], axis=1)  # [D, TT]
        # pre-tile to [128, dc*TT + t] so each DMA is contiguous per partition
        xt_r = np.ascontiguousarray(
            xt_r.reshape(DC, 128, TT).transpose(1, 0, 2).reshape(128, DC * TT)
        )
        comp_wt_r = comp_w[r * OSL : (r + 1) * OSL].T            # [F, OSL]
        comp_wt_r = np.ascontiguousarray(
            comp_wt_r.reshape(FC, 128, OSL).transpose(1, 0, 2).reshape(128, FC * OSL)
        )
        comp_b_r = np.ascontiguousarray(comp_b[r * OSL : (r + 1) * OSL].reshape(1, OSL))
        in_maps.append(
            {
                "xt": xt_r,
                "pca_wt": pca_wt,
                "pca_w": pca_w,
                "pca_b": pca_b_row,
                "pca_b_col": pca_b_col,
                "conv_wt": conv_wt,
                "conv_b_bc": conv_b_bc,
                "cent": cent,
                "comp_wt": comp_wt_r,
                "comp_b": comp_b_r,
            }
        )

    res = run_bass_kernel_spmd(nc, in_maps, core_ids=list(range(N_CORES)))
    kernel.last_results = res
    out = np.empty((N, OUT), dtype=np.float32)
    for r in range(N_CORES):
        out[:, r * OSL : (r + 1) * OSL] = np.asarray(res.results[r]["out"])
    return out


# revision 38
# speedup vs baseline: 1.1235x; 1.1235x over previous
"""AnyLoc/NetVLAD pooling kernel for 8 Trainium2 NeuronCores.

Full inputs in, full output out. Internally:
  - data-parallel over batch: core r owns samples {2r, 2r+1}
  - comp_w sharded over its OUT dim: core r owns output columns [256r, 256r+256)
  - per-sample AllGathers of the tiny VLAD vectors (overlapped with compute),
    final row-norm via a 64-byte AllReduce; host concatenates output slices.

Hardcoded problem shape: N=16, T=2048, D=1024, P=256, K=64, OUT=2048 (f32).
"""

import math
import sys
import types

import numpy as np

N_CORES = 8
N, T, D, P, K, OUT = 16, 2048, 1024, 256, 64, 2048
SPC = N // N_CORES          # samples per core = 2
TT = SPC * T                # tokens per core = 4096
NT = TT // 128              # 128-token tiles per core = 32
NTS = T // 128              # tiles per sample = 16
OSL = OUT // N_CORES        # output slice per core = 256
F = K * P                   # flattened VLAD dim = 16384
FC = F // 128               # f-chunks = 128
DC = D // 128               # d-chunks = 8
PC = P // 128               # p-chunks = 2
HZ = P + K                  # fused h|z matmul width = 320


def _install_ntff_hook():
    """Make run_bass_kernel_spmd(trace=True) usable in this container: the
    image's antenv stub lacks axon_hooks, so inject one wired to the axon .so.
    Harmless if tracing is never requested."""
    if "antenv.axon_hooks" in sys.modules:
        return
    try:
        from trn_agent_boot.trn_boot import _ntff_profile_via_ctypes

        hook = _ntff_profile_via_ctypes("/opt/axon/libaxon_pjrt.so")
    except Exception:
        hook = None
    mod = types.ModuleType("antenv.axon_hooks")
    mod.get_axon_ntff_profile_hook = lambda: hook
    mod.set_axon_ntff_profile_hook = lambda h: None
    sys.modules["antenv.axon_hooks"] = mod


_NC_CACHE = {}


def _build():
    import concourse.bacc as bacc
    import concourse.mybir as mybir
    import concourse.tile as tile
    from concourse.masks import make_identity

    f32 = mybir.dt.float32
    bf16 = mybir.dt.bfloat16
    Alu = mybir.AluOpType
    Act = mybir.ActivationFunctionType

    nc = bacc.Bacc(
        "TRN2",
        target_bir_lowering=False,
        debug=False,
        enable_asserts=False,
        num_devices=N_CORES,
    )

    # ---- DRAM I/O (per-core shards; names are the in_map keys) ----
    xt_d = nc.dram_tensor("xt", [128, DC * TT], f32, kind="ExternalInput")
    pca_wt_d = nc.dram_tensor("pca_wt", [D, P], f32, kind="ExternalInput")
    pca_w_d = nc.dram_tensor("pca_w", [P, D], f32, kind="ExternalInput")
    pca_b_d = nc.dram_tensor("pca_b", [1, P], f32, kind="ExternalInput")
    pca_bc_d = nc.dram_tensor("pca_b_col", [128, 2], f32, kind="ExternalInput")
    conv_wt_d = nc.dram_tensor("conv_wt", [P, K], f32, kind="ExternalInput")
    conv_bb_d = nc.dram_tensor("conv_b_bc", [128, K], f32, kind="ExternalInput")
    cent_d = nc.dram_tensor("cent", [K, P], f32, kind="ExternalInput")
    comp_wt_d = nc.dram_tensor("comp_wt", [128, FC * OSL], f32, kind="ExternalInput")
    comp_b_d = nc.dram_tensor("comp_b", [1, OSL], f32, kind="ExternalInput")
    out_d = nc.dram_tensor("out", [N, OSL], f32, kind="ExternalOutput")

    rg = [list(range(N_CORES))]
    LN8TH = math.log(0.125)

    with tile.TileContext(nc) as tc:
        with (
            tc.tile_pool(name="consts", bufs=1) as consts,
            tc.tile_pool(name="work", bufs=3) as work,
            tc.tile_pool(name="small", bufs=4) as small,
            tc.tile_pool(name="ph", bufs=2, space="PSUM") as ph_pool,
            tc.tile_pool(name="pagg", bufs=2, space="PSUM") as pagg_pool,
            tc.tile_pool(name="pmisc", bufs=2, space="PSUM") as pmisc_pool,
            tc.tile_pool(name="pout", bufs=2, space="PSUM") as pout_pool,
            tc.tile_pool(name="dram", bufs=1, space="DRAM") as dram,
        ):
            # ---- persistent SBUF tensors ----
            WG_sb = consts.tile([128, DC, HZ], bf16, tag="WG")    # [pca_w.T | G]
            pw_sb = consts.tile([128, PC, D], bf16, tag="pw")     # pca_w
            cwt_sb = consts.tile([128, PC, K], bf16, tag="cwt")   # conv_w.T
            cbb_sb = consts.tile([128, K], f32, tag="cbb")        # conv_b bcast
            cent_sb = consts.tile([K, P], f32, tag="cent")
            bg_sb = consts.tile([1, HZ], bf16, tag="bg")          # [pca_b | g0]
            pcabc_sb = consts.tile([128, 2], bf16, tag="pcabc")
            compb_sb = consts.tile([1, OSL], bf16, tag="compb")
            ones_sb = consts.tile([1, 128], bf16, tag="ones")
            ident_sb = consts.tile([128, 128], f32, tag="ident")
            xt_sb = consts.tile([128, DC, TT], bf16, tag="xt")
            cwT_sb = consts.tile([128, FC, OSL], bf16, tag="cwT")  # comp_w.T
            h_all = consts.tile([128, NT, P + 1], bf16, tag="hall")
            z_all = consts.tile([128, NT, K], f32, tag="zall")
            u_all = consts.tile([128, NT, K], bf16, tag="uall")
            nsq_all = consts.tile([128, NT], f32, tag="nsq")
            inv_all = consts.tile([128, NT], f32, tag="inv")
            S_all = consts.tile([128, NT], f32, tag="Sall")
            i32 = mybir.dt.int32
            mg_sb = consts.tile([128, NT], i32, tag="mg")
            it_sb = consts.tile([128, NT + 4], i32, tag="itsb")
            rt_sb = consts.tile([128, NT + 4], f32, tag="rtsb")
            rS_all = consts.tile([128, NT], f32, tag="rSall")
            vT_own = consts.tile([128, SPC, 128], bf16, tag="vTown")
            vT_ev = consts.tile([128, N_CORES, 128], bf16, tag="vTev")
            vT_od = consts.tile([128, N_CORES, 128], bf16, tag="vTod")
            sq_scr = consts.tile([128, P], bf16, tag="sqscr")
            sq64_scr = consts.tile([K, P], bf16, tag="sq64")
            osq_sb = consts.tile([N, 1], f32, tag="osq")
            rno_sb = consts.tile([N, 1], f32, tag="rno")
            rno_od = consts.tile([N_CORES, 1], f32, tag="rnood")

            # DRAM bounce buffers for collectives (f32-typed views of bf16
            # bits: halves the CCE element count -> faster AllGather)
            agv_in = [
                dram.tile([128, 64], f32, tag=f"agi{s}", name=f"agv_in{s}")
                for s in range(SPC)
            ]
            agv_out = [
                dram.tile(
                    [128 * N_CORES, 64], f32, tag=f"ago{s}", name=f"agv_out{s}"
                )
                for s in range(SPC)
            ]
            ar_in = dram.tile([N, 1], f32, tag="ari")
            ar_out = dram.tile([N, 1], f32, tag="aro")

            # ---- const loads ----
            nc.gpsimd.dma_start(
                WG_sb[:, :, 0:P], pca_wt_d.ap().rearrange("(c k) p -> k c p", k=128)
            )
            nc.gpsimd.dma_start(
                pw_sb[:], pca_w_d.ap().rearrange("(c k) d -> k c d", k=128)
            )
            nc.gpsimd.dma_start(
                cwt_sb[:], conv_wt_d.ap().rearrange("(c k) n -> k c n", k=128)
            )
            nc.sync.dma_start(cbb_sb[:], conv_bb_d.ap())
            nc.sync.dma_start(cent_sb[:], cent_d.ap())
            nc.gpsimd.dma_start(bg_sb[0:1, 0:P], pca_b_d.ap())
            nc.gpsimd.dma_start(pcabc_sb[:], pca_bc_d.ap())
            nc.gpsimd.dma_start(compb_sb[:], comp_b_d.ap())
            nc.vector.memset(ones_sb[:], 1.0)
            nc.vector.memset(mg_sb[:], 0x5F3759DF)
            make_identity(nc, ident_sb[:])

            def rsqrt_dve(out_ap, in_ap, scol, width, rows=128):
                """out = 1/sqrt(in) on VectorE only (bit trick + 2 Newton
                steps, ~5e-6 rel err) - avoids ACT table-set switching."""
                ti = it_sb[0:rows, scol : scol + width]
                tm = rt_sb[0:rows, scol : scol + width]
                mg = mg_sb[0:rows, 0:width]
                nc.vector.tensor_scalar(
                    ti, in_ap.bitcast(i32), 1, None, op0=Alu.logical_shift_right
                )
                nc.vector.scalar_tensor_tensor(
                    out_ap.bitcast(i32), ti, -1, mg, op0=Alu.mult, op1=Alu.add
                )
                for _ in range(2):
                    nc.vector.tensor_mul(tm, in_ap, out_ap)
                    nc.vector.tensor_mul(tm, tm, out_ap)
                    nc.vector.tensor_scalar(
                        tm, tm, -0.5, 1.5, op0=Alu.mult, op1=Alu.add
                    )
                    nc.vector.tensor_mul(out_ap, out_ap, tm)

            # ---- x loads: token-quarter major so PE can start early.
            # Host pre-tiles xt to [128, dc*TT + t]: every DMA below is one
            # contiguous run per partition (minimal SWDGE descriptor count).
            # Three staged phases with pairwise dep edges so each phase gets
            # the full HBM bandwidth instead of ring round-robin: x(sample 0),
            # then x(sample 1), then comp_w.T.
            from concourse.tile_rust import add_dep_helper

            NQ = 2
            QW = TT // NQ  # one sample's tokens per load block
            xt_re = xt_d.ap().rearrange("k (c t) -> k c t", c=DC)
            xdmas = [[], []]
            for q in range(NQ):
                for dc in range(DC):
                    i = nc.gpsimd.dma_start(
                        xt_sb[:, dc, q * QW : (q + 1) * QW],
                        xt_re[:, dc, q * QW : (q + 1) * QW],
                    )
                    xdmas[q].append(i)
                    if q == 1:
                        add_dep_helper(
                            i.ins, xdmas[0][dc].ins, reason="stage x1 after x0"
                        )
            # ---- comp_w.T stream-in (host pre-tiled the same way) ----
            for g in range(8):
                i = nc.gpsimd.dma_start(
                    cwT_sb[:, g * 16 : (g + 1) * 16, :],
                    comp_wt_d.ap()[:, g * 16 * OSL : (g + 1) * 16 * OSL],
                )
                add_dep_helper(
                    i.ins, xdmas[1][g].ins, reason="stage comp_w after x1"
                )

            # ---- G = pca_w.T @ conv_w.T  [D, K], g0 = pca_b @ conv_w.T ----
            for dc in range(DC):
                pG = pmisc_pool.tile([128, 128], f32, tag="pm")
                for pc in range(PC):
                    nc.tensor.matmul(
                        pG[:, 0:K],
                        pw_sb[:, pc, dc * 128 : (dc + 1) * 128],
                        cwt_sb[:, pc, :],
                        start=(pc == 0),
                        stop=(pc == PC - 1),
                    )
                nc.vector.tensor_copy(WG_sb[:, dc, P:HZ], pG[:, 0:K])
            pg0 = pmisc_pool.tile([128, 128], f32, tag="pm")
            for pc in range(PC):
                nc.tensor.matmul(
                    pg0[0:1, 0:K],
                    pcabc_sb[:, pc : pc + 1],
                    cwt_sb[:, pc, :],
                    start=(pc == 0),
                    stop=(pc == PC - 1),
                )
            nc.vector.tensor_copy(bg_sb[0:1, P:HZ], pg0[0:1, 0:K])

            # ================= per-sample stages, manually interleaved ======
            def pass_a(s):
                """Fused h|z matmuls + copies + row sum-squares for sample s."""
                for ti in range(NTS):
                    t0 = s * T + ti * 128
                    g = s * NTS + ti
                    ph = ph_pool.tile([128, HZ], f32, tag="ph")
                    for dc in range(DC):
                        nc.tensor.matmul(
                            ph[:],
                            xt_sb[:, dc, t0 : t0 + 128],
                            WG_sb[:, dc, :],
                            start=(dc == 0),
                            stop=False,
                        )
                    nc.tensor.matmul(
                        ph[:], ones_sb[0:1, 0:128], bg_sb[:], start=False, stop=True
                    )
                    nc.vector.tensor_copy(h_all[:, g, 0:P], ph[:, 0:P])
                    nc.vector.tensor_copy(z_all[:, g, :], ph[:, P:HZ])
                    nc.vector.scalar_tensor_tensor(
                        sq_scr[:],
                        h_all[:, g, 0:P],
                        1.0,
                        h_all[:, g, 0:P],
                        op0=Alu.mult,
                        op1=Alu.mult,
                        accum_out=nsq_all[:, g : g + 1],
                    )

            def pass_b(s):
                """Batched norms, softmax weights, VLAD agg + gather for s."""
                s16 = slice(s * NTS, (s + 1) * NTS)
                rsqrt_dve(inv_all[:, s16], nsq_all[:, s16], s * NTS, NTS)
                # n_t column = nsq * (1/n) = n
                nc.vector.tensor_mul(
                    h_all[:, s16, P], nsq_all[:, s16], inv_all[:, s16]
                )
                for ti in range(NTS):
                    g = s * NTS + ti
                    nc.vector.scalar_tensor_tensor(
                        z_all[:, g, :],
                        z_all[:, g, :],
                        inv_all[:, g : g + 1],
                        cbb_sb[:],
                        op0=Alu.mult,
                        op1=Alu.add,
                    )
                nc.scalar.activation(u_all[:, s16, :], z_all[:, s16, :], Act.Exp)
                nc.vector.reduce_sum(
                    S_all[:, s16], u_all[:, s16, :], axis=mybir.AxisListType.X
                )
                nc.vector.reciprocal(rS_all[:, s16], S_all[:, s16])
                agg_t = pagg_pool.tile([K, P + 1], f32, tag="agg")
                for ti in range(NTS):
                    g = s * NTS + ti
                    w_t = work.tile([128, K], bf16, tag="w")
                    nc.vector.tensor_scalar(
                        w_t[:],
                        u_all[:, g, :],
                        rS_all[:, g : g + 1],
                        inv_all[:, g : g + 1],
                        op0=Alu.mult,
                        op1=Alu.mult,
                    )
                    nc.tensor.matmul(
                        agg_t[:],
                        w_t[:],
                        h_all[:, g, :],
                        start=(ti == 0),
                        stop=(ti == NTS - 1),
                    )

                # per-sample VLAD: subtract centroids, intra-normalize
                agg_sb = work.tile([K, P + 1], f32, tag="aggsb")
                nc.vector.tensor_copy(agg_sb[:], agg_t[:])
                vlneg = work.tile([K, P], f32, tag="vlneg")
                nc.vector.scalar_tensor_tensor(
                    vlneg[:],
                    cent_sb[:],
                    agg_sb[:, P : P + 1],
                    agg_sb[:, 0:P],
                    op0=Alu.mult,
                    op1=Alu.subtract,
                )
                vsq = small.tile([K, 1], f32, tag="vsq")
                nc.vector.scalar_tensor_tensor(
                    sq64_scr[:],
                    vlneg[:],
                    1.0,
                    vlneg[:],
                    op0=Alu.mult,
                    op1=Alu.mult,
                    accum_out=vsq[:],
                )
                r_t = small.tile([K, 1], f32, tag="r")
                rsqrt_dve(r_t[:], vsq[:], NT + s, 1, rows=K)
                vn = work.tile([K, P], f32, tag="vn")
                nc.vector.tensor_scalar(
                    vn[:], vlneg[:], r_t[:], -0.125, op0=Alu.mult, op1=Alu.mult
                )
                v128 = work.tile([128, 128], f32, tag="v128")
                nc.scalar.dma_start(v128[0 : 2 * K : 2, :], vn[:, 0:128])
                nc.scalar.dma_start(v128[1 : 2 * K : 2, :], vn[:, 128:256])
                ptr = pmisc_pool.tile([128, 128], f32, tag="pm")
                nc.tensor.transpose(ptr[:], v128[:], ident_sb[:])
                nc.vector.tensor_copy(vT_own[:, s, :], ptr[:])

                # AllGather this sample's VLAD across cores (as f32 bits)
                nc.sync.dma_start(agv_in[s][:], vT_own[:, s, :].bitcast(f32))
                nc.gpsimd.collective_compute(
                    "AllGather",
                    Alu.bypass,
                    replica_groups=rg,
                    ins=[agv_in[s].opt()],
                    outs=[agv_out[s].opt()],
                )
                vT_dst = vT_ev if s == 0 else vT_od
                nc.sync.dma_start(
                    vT_dst[:],
                    agv_out[s][:].bitcast(bf16).rearrange(
                        "(b k) c -> k b c", b=N_CORES
                    ),
                )

            pout_t = [None, None]

            def gemm_half(s):
                """out rows of parity s for this core's 256 output columns."""
                vT_dst = vT_ev if s == 0 else vT_od
                pout_t[s] = pout_pool.tile(
                    [N_CORES, OSL], f32, tag="po", name=f"pout{s}"
                )
                for c in range(FC):
                    nc.tensor.matmul(
                        pout_t[s][:],
                        vT_dst[:, :, c],
                        cwT_sb[:, c, :],
                        start=(c == 0),
                        stop=False,
                    )
                nc.tensor.matmul(
                    pout_t[s][:],
                    ones_sb[0:1, 0:N_CORES],
                    compb_sb[:],
                    start=False,
                    stop=True,
                )

            pass_a(0)
            pass_b(0)          # ends with AllGather of sample-0 VLADs
            pass_a(1)          # PE busy here while AG0 completes
            pass_b(1)
            gemm_half(0)
            gemm_half(1)

            # ---- final row norm: AllReduce partial sum-squares (64 B) ----
            out_sl = [None, None]
            for s in range(SPC):
                out_sl[s] = work.tile(
                    [N_CORES, OSL], f32, tag=f"osl{s}", name=f"out_sl{s}"
                )
                nc.vector.tensor_copy(out_sl[s][:], pout_t[s][:])
                osqp = small.tile([N_CORES, 1], f32, tag=f"osqp{s}")
                nc.vector.scalar_tensor_tensor(
                    sq_scr[0:N_CORES, 0:OSL],
                    out_sl[s][:],
                    1.0,
                    out_sl[s][:],
                    op0=Alu.mult,
                    op1=Alu.mult,
                    accum_out=osqp[:],
                )
                nc.sync.dma_start(ar_in[s * N_CORES : (s + 1) * N_CORES, :], osqp[:])
            nc.gpsimd.collective_compute(
                "AllReduce",
                Alu.add,
                replica_groups=rg,
                ins=[ar_in.opt()],
                outs=[ar_out.opt()],
            )
            nc.sync.dma_start(osq_sb[:], ar_out[:])
            rsqrt_dve(rno_sb[:], osq_sb[:], NT + 2, 1, rows=N)
            nc.sync.dma_start(rno_od[:], rno_sb[N_CORES:N, :])
            for s in range(SPC):
                scal = rno_sb[0:N_CORES, :] if s == 0 else rno_od[:]
                of = work.tile([N_CORES, OSL], f32, tag=f"of{s}")
                nc.vector.tensor_scalar(
                    of[:], out_sl[s][:], scal, None, op0=Alu.mult
                )
                nc.sync.dma_start(out_d.ap()[s : N : SPC, :], of[:])

    nc.compile()
    return nc


def _get_nc():
    if "nc" not in _NC_CACHE:
        _install_ntff_hook()
        _NC_CACHE["nc"] = _build()
    return _NC_CACHE["nc"]


def kernel(**inputs):
    x = np.asarray(inputs["x"], dtype=np.float32)
    pca_w = np.asarray(inputs["pca_w"], dtype=np.float32)
    pca_b = np.asarray(inputs["pca_b"], dtype=np.float32)
    conv_w = np.asarray(inputs["conv_w"], dtype=np.float32)
    conv_b = np.asarray(inputs["conv_b"], dtype=np.float32)
    cent = np.asarray(inputs["centroids"], dtype=np.float32)
    comp_w = np.asarray(inputs["comp_w"], dtype=np.float32)
    comp_b = np.asarray(inputs["comp_b"], dtype=np.float32)

    nc = _get_nc()
    from concourse.bass_utils import run_bass_kernel_spmd

    # host-side layout prep (pure slicing / transposition)
    pca_wt = np.ascontiguousarray(pca_w.T)                      # [D, P]
    pca_b_row = np.ascontiguousarray(pca_b.reshape(1, P))
    pca_b_col = np.ascontiguousarray(pca_b.reshape(2, 128).T)   # [128, 2]
    conv_wt = np.ascontiguousarray(conv_w.T)                    # [P, K]
    conv_b_bc = np.ascontiguousarray(np.broadcast_to(conv_b, (128, K)))
    xt = x.transpose(0, 2, 1)                                   # [N, D, T]

    in_maps = []
    for r in range(N_CORES):
        xt_r = np.concatenate([xt[SPC * r + j] for j in range(SPC)], axis=1)  # [D, TT]
        # pre-tile to [128, dc*TT + t] so each DMA is contiguous per partition
        xt_r = np.ascontiguousarray(
            xt_r.reshape(DC, 128, TT).transpose(1, 0, 2).reshape(128, DC * TT)
        )
        comp_wt_r = comp_w[r * OSL : (r + 1) * OSL].T            # [F, OSL]
        comp_wt_r = np.ascontiguousarray(
            comp_wt_r.reshape(FC, 128, OSL).transpose(1, 0, 2).reshape(128, FC * OSL)
        )
        comp_b_r = np.ascontiguousarray(comp_b[r * OSL : (r + 1) * OSL].reshape(1, OSL))
        in_maps.append(
            {
                "xt": xt_r,
                "pca_wt": pca_wt,
                "pca_w": pca_w,
                "pca_b": pca_b_row,
                "pca_b_col": pca_b_col,
                "conv_wt": conv_wt,
                "conv_b_bc": conv_b_bc,
                "cent": cent,
                "comp_wt": comp_wt_r,
                "comp_b": comp_b_r,
            }
        )

    res = run_bass_kernel_spmd(nc, in_maps, core_ids=list(range(N_CORES)))
    kernel.last_results = res
    out = np.empty((N, OUT), dtype=np.float32)
    for r in range(N_CORES):
        out[:, r * OSL : (r + 1) * OSL] = np.asarray(res.results[r]["out"])
    return out
